# revision 16
# baseline (speedup 1.0000x reference)
"""Expert-parallel MoE (top-k routing + SwiGLU experts) for 8 Trainium2 cores.

Strategy
--------
- Host computes the (tiny) gate: logits = x @ gate_w (+ noise * noise_weight),
  top-k selection, sparse softmax weights.  0.03% of total FLOPs.
- Expert-parallel: core e owns expert e's weights.  Host gathers the tokens
  routed to expert e (padded to a common capacity C), core e runs a dense
  fused SwiGLU MLP over them:  out = (x@w1+b1) * silu(x@w2+b2) @ wp + bp,
  scaled by the per-token gate weight (folded into the final evacuation).
- Host scatter-adds the 8 partial outputs back to token positions.

Device kernel v2 (tokens on the free axis; bf16 matmul inputs, f32 PSUM):
- x^T [D,C] bf16 and the full wp [H,D] bf16 (8MB) stay resident in SBUF.
- Token-block OUTER loop (blocks of 512 = one PSUM bank):
    for g in 8 h-groups of 512 rows (w1g/w2g streamed, double-buffered):
      for hj in 4:  ps2 = w2g.T@x (8-MM chain), ps1 = w1g.T@x (8-MM chain)
                    s2 = silu(ps2 + b2)  [ACT] ; ht = (ps1 + b1) * s2  [DVE]
    for dm in 8:  psB = sum over all 32 h-tiles of wp.T @ ht  (32-MM chain)
                  ot = (psB + bp) * gate_weight  -> bf16 -> DMA out
  The per-block epilogue overlaps the next block's h-phase on the PE;
  out-chains consume ht tiles in production order so the ACT/DVE lag is
  hidden under the chain's own latency.
- Output is bf16 (halves store traffic); host converts to f32.
"""

import sys
import numpy as np

sys.path.insert(0, "/opt/trn_rl_repo")

D = 1024
H = 4096
E = 8
KD = D // 128          # 8 k-tiles over D
G = 8                  # h-groups of 512 rows
HJ = 4                 # 128-row h-tiles per group (G*HJ*128 == H)
NHT = G * HJ           # 32 h-tiles total
TB = 512               # token block (matmul output must fit one PSUM bank)

_NC_CACHE = {}


def _build(C):
    import concourse.mybir as mybir
    import concourse.tile as tile
    from concourse import bacc

    f32 = mybir.dt.float32
    bf16 = mybir.dt.bfloat16
    ACT = mybir.ActivationFunctionType
    ALU = mybir.AluOpType

    nc = bacc.Bacc()
    xeT = nc.dram_tensor("xeT", [D, C], bf16, kind="ExternalInput")
    w1 = nc.dram_tensor("w1", [D, H], bf16, kind="ExternalInput")
    w2 = nc.dram_tensor("w2", [D, H], bf16, kind="ExternalInput")
    wp = nc.dram_tensor("wp", [H, D], bf16, kind="ExternalInput")
    b1 = nc.dram_tensor("b1", [H], f32, kind="ExternalInput")
    b2 = nc.dram_tensor("b2", [H], f32, kind="ExternalInput")
    bp = nc.dram_tensor("bp", [D], f32, kind="ExternalInput")
    gwb = nc.dram_tensor("gwb", [128, C], f32, kind="ExternalInput")
    outT = nc.dram_tensor("outT", [D, C], bf16, kind="ExternalOutput")

    # Token blocks (all <= TB = one PSUM bank of f32).  The FIRST block is
    # full-width: its h-phase paces weight demand slowest, which matters
    # while x/wp still stream in.  The rest are balanced (>=256) so no
    # block's h-phase outruns the steady weight stream.
    if C <= TB:
        blocks = [(0, C)]
    else:
        nrem = (C - TB + TB - 1) // TB
        base = ((C - TB + nrem - 1) // nrem + 3) // 4 * 4
        blocks = [(0, TB)]
        o = TB
        while o < C:
            blocks.append((o, min(base, C - o)))
            o += base

    # strided views
    xTr = xeT.rearrange("(kt p) c -> kt p c", p=128)              # [8,128,C]
    # w1/w2 group tiles: [gg, p, k, c] with h-col = gg*512 + c
    w1r = w1.rearrange("(k p) (gg c) -> gg p k c", p=128, c=512)
    w2r = w2.rearrange("(k p) (gg c) -> gg p k c", p=128, c=512)
    # wp tiles: rows h = gg*512 + hk*128 + p; columns d contiguous
    wpr = wp.rearrange("(gg hk p) c -> gg p hk c", p=128, hk=4)
    b1r = b1.rearrange("(m p) -> p m", p=128)                     # [128,32]
    b2r = b2.rearrange("(m p) -> p m", p=128)
    bpr = bp.rearrange("(m p) -> p m", p=128)                     # [128,8]

    with tile.TileContext(nc) as tc:
        with (
            tc.tile_pool(name="pw1", bufs=3) as pw1,
            tc.tile_pool(name="pw2", bufs=3) as pw2,
            tc.tile_pool(name="pwp", bufs=1) as pwp,
            tc.tile_pool(name="px", bufs=1) as px,
            tc.tile_pool(name="pht", bufs=1) as pht,
            tc.tile_pool(name="ps2", bufs=3) as ps2p,
            tc.tile_pool(name="pot", bufs=4) as pot,
            tc.tile_pool(name="pgw", bufs=1) as pgw,
            tc.tile_pool(name="pb", bufs=1) as pb,
            tc.tile_pool(name="pp", bufs=4, space="PSUM") as pp,
            tc.tile_pool(name="po", bufs=4, space="PSUM") as po,
        ):
            b1s = pb.tile([128, NHT], f32, tag="b1s")
            nc.gpsimd.dma_start(b1s[:], b1r)
            b2s = pb.tile([128, NHT], f32, tag="b2s")
            nc.gpsimd.dma_start(b2s[:], b2r)
            bps = pb.tile([128, KD], f32, tag="bps")
            nc.gpsimd.dma_start(bps[:], bpr)

            # resident x^T tiles, filled block-column-wise: block 0's columns
            # land first (subtile deps unblock its matmuls early); block b+1's
            # columns stream during block b's h-phase, interleaved with w1.
            xk = [px.tile([128, C], bf16, tag=f"x{kt}", name=f"x{kt}")
                  for kt in range(KD)]
            bo0, bs0 = blocks[0]
            for kt in range(2):
                nc.sync.dma_start(xk[kt][:, bo0:bo0 + bs0],
                                  xTr[kt][:, bo0:bo0 + bs0])

            # resident wp (bf16, 8MB): chunks queued on the scalar HWDGE ring
            # BEHIND block 0's w2 stream (FIFO) so they don't steal startup BW
            wps = pwp.tile([128, NHT * 1024], bf16, tag="wps", name="wps")

            # gate weights, broadcast to 128 partitions host-side; loaded
            # during block 1 (first needed by block 0's epilogue)
            gwt = pgw.tile([128, C], f32, tag="gw")

            # ht tiles: one slot per (g,hj); WAR deps across blocks resolve
            # in PE-FIFO order with zero stalls.
            hts = [pht.tile([128, TB], bf16, tag=f"ht{i}", name=f"ht{i}")
                   for i in range(NHT)]

            def _wp_chunk(g):
                eng = nc.sync if g % 2 == 0 else nc.scalar
                eng.dma_start(
                    wps[:, g * HJ * 1024:(g + 1) * HJ * 1024]
                    .rearrange("p (hk c) -> p hk c", c=1024),
                    wpr[g])

            for bi, (bo, bs) in enumerate(blocks):
                # ---- h-phase: stream w1/w2 by group, produce 32 ht tiles --
                for g in range(G):
                    w1g = pw1.tile([128, KD * 512], bf16, tag="w1s")
                    w1v = w1g[:].rearrange("p (k c) -> p k c", c=512)
                    if bi == 0 and g == 0:
                        # split the very first loads so subtile deps unblock
                        # the first chains ASAP; interleave the remaining x
                        # chunks with the w1 k-splits in consumption order
                        for i in range(4):
                            nc.sync.dma_start(w1v[:, 2 * i:2 * i + 2],
                                              w1r[g][:, 2 * i:2 * i + 2])
                            for kt in (2 * i + 2, 2 * i + 3):
                                if kt < KD:
                                    nc.sync.dma_start(
                                        xk[kt][:, bo0:bo0 + bs0],
                                        xTr[kt][:, bo0:bo0 + bs0])
                    else:
                        nc.sync.dma_start(w1v, w1r[g])
                    if bi + 1 < len(blocks):
                        # next block's x columns, k-tile g, behind w1g (FIFO)
                        nbo, nbs = blocks[bi + 1]
                        nc.sync.dma_start(xk[g][:, nbo:nbo + nbs],
                                          xTr[g][:, nbo:nbo + nbs])
                    w2g = pw2.tile([128, KD * 512], bf16, tag="w2s")
                    w2v = w2g[:].rearrange("p (k c) -> p k c", c=512)
                    if bi == 0 and g == 0:
                        for kh in range(0, KD, 2):
                            nc.scalar.dma_start(w2v[:, kh:kh + 2],
                                                w2r[g][:, kh:kh + 2])
                    else:
                        nc.scalar.dma_start(w2v, w2r[g])
                    if bi == 0 and g >= 1:
                        _wp_chunk(g - 1)
                        if g == 4:
                            # emitted before block 0's epilogue (its reader),
                            # queued behind half of block 0's w1 stream
                            nc.sync.dma_start(gwt[:], gwb[:])
                        if g == G - 1:
                            _wp_chunk(G - 1)

                    for hj in range(HJ):
                        hm = g * HJ + hj
                        co = hj * 128
                        # gate chain first so ACT starts a chain earlier
                        psA = pp.tile([128, bs], f32, tag="ps")
                        for k in range(KD):
                            nc.tensor.matmul(
                                psA[:], w2g[:, k * 512 + co: k * 512 + co + 128],
                                xk[k][:, bo:bo + bs],
                                start=(k == 0), stop=(k == KD - 1))
                        psB = pp.tile([128, bs], f32, tag="ps")
                        for k in range(KD):
                            nc.tensor.matmul(
                                psB[:], w1g[:, k * 512 + co: k * 512 + co + 128],
                                xk[k][:, bo:bo + bs],
                                start=(k == 0), stop=(k == KD - 1))
                        s2 = ps2p.tile([128, bs], f32, tag="s2")
                        nc.scalar.activation(s2[:], psA[:], ACT.Silu,
                                             bias=b2s[:, hm:hm + 1])
                        nc.vector.scalar_tensor_tensor(
                            hts[hm][:, :bs], psB[:], b1s[:, hm:hm + 1], s2[:],
                            op0=ALU.add, op1=ALU.mult)

                # ---- out-phase: 32-MM chains over all h, then epilogue ----
                for dm in range(KD):
                    psO = po.tile([128, bs], f32, tag="po")
                    n = 0
                    for g in range(G):
                        for hk in range(HJ):
                            nc.tensor.matmul(
                                psO[:],
                                wps[:, (g * HJ + hk) * 1024 + dm * 128:
                                    (g * HJ + hk) * 1024 + dm * 128 + 128],
                                hts[g * HJ + hk][:, :bs],
                                start=(n == 0), stop=(n == NHT - 1))
                            n += 1
                    ot = pot.tile([128, TB], bf16, tag="ot")
                    nc.vector.scalar_tensor_tensor(
                        ot[:, :bs], psO[:], bps[:, dm:dm + 1],
                        gwt[:, bo:bo + bs], op0=ALU.add, op1=ALU.mult)
                    nc.sync.dma_start(
                        outT[dm * 128:(dm + 1) * 128, bo:bo + bs], ot[:, :bs])

    nc.finalize()
    return nc


def _route(x2d, noise2d, gate_w, noise_weight, kk):
    T = x2d.shape[0]
    logits = x2d @ gate_w
    logits = logits + noise2d * noise_weight[None, :]
    kk = int(kk)
    Ee = logits.shape[1]
    if kk >= Ee:
        sel = np.ones((T, Ee), dtype=bool)
    else:
        part = np.argpartition(-logits, kk - 1, axis=1)[:, :kk]
        sel = np.zeros((T, Ee), dtype=bool)
        sel[np.arange(T)[:, None], part] = True
    mx = logits.max(axis=1, keepdims=True)
    ex = np.exp(logits - mx, dtype=np.float32) * sel
    gw = ex / ex.sum(axis=1, keepdims=True)
    return sel, gw.astype(np.float32)


def _prep_maps(x2d, gw, idxs, C, w1, b1, w2, b2, wp, bp):
    import ml_dtypes
    bf16 = ml_dtypes.bfloat16
    in_maps = []
    for e in range(E):
        idx = idxs[e]
        n = len(idx)
        xeT = np.zeros((D, C), dtype=bf16)
        xeT[:, :n] = x2d[idx].T.astype(bf16)
        gwb = np.zeros((128, C), dtype=np.float32)
        gwb[:, :n] = gw[idx, e][None, :]
        in_maps.append({
            "xeT": xeT,
            "w1": np.ascontiguousarray(w1[e]).astype(bf16),
            "w2": np.ascontiguousarray(w2[e]).astype(bf16),
            "wp": np.ascontiguousarray(wp[e]).astype(bf16),
            "b1": np.ascontiguousarray(b1[e], dtype=np.float32),
            "b2": np.ascontiguousarray(b2[e], dtype=np.float32),
            "bp": np.ascontiguousarray(bp[e], dtype=np.float32),
            "gwb": gwb,
        })
    return in_maps


def kernel(**inputs):
    from concourse.bass_utils import run_bass_kernel_spmd

    x = np.asarray(inputs["x"], dtype=np.float32)
    noise = np.asarray(inputs["noise"], dtype=np.float32)
    gate_w = np.asarray(inputs["gate_w"], dtype=np.float32)
    noise_weight = np.asarray(inputs["noise_weight"], dtype=np.float32)
    w1 = np.asarray(inputs["w1"], dtype=np.float32)
    b1 = np.asarray(inputs["b1"], dtype=np.float32)
    w2 = np.asarray(inputs["w2"], dtype=np.float32)
    b2 = np.asarray(inputs["b2"], dtype=np.float32)
    wp = np.asarray(inputs["wp"], dtype=np.float32)
    bp = np.asarray(inputs["bp"], dtype=np.float32)
    kk = int(np.asarray(inputs["k"]))

    B, S, _ = x.shape
    T = B * S
    x2d = np.ascontiguousarray(x.reshape(T, D))
    noise2d = noise.reshape(T, E)

    sel, gw = _route(x2d, noise2d, gate_w, noise_weight, kk)
    idxs = [np.nonzero(sel[:, e])[0] for e in range(E)]
    maxn = max(len(i) for i in idxs)
    C = max(512, ((maxn + 3) // 4) * 4)

    if C not in _NC_CACHE:
        _NC_CACHE[C] = _build(C)
    nc = _NC_CACHE[C]

    in_maps = _prep_maps(x2d, gw, idxs, C, w1, b1, w2, b2, wp, bp)
    res = run_bass_kernel_spmd(nc, in_maps, core_ids=list(range(E))).results

    y2d = np.zeros((T, D), dtype=np.float32)
    for e in range(E):
        n = len(idxs[e])
        if n:
            y2d[idxs[e]] += res[e]["outT"][:, :n].T.astype(np.float32)
    return y2d.reshape(B, S, D)
